# revision 45
# baseline (speedup 1.0000x reference)
"""Masked per-protein attention (sparse_attention) on 8 trn2 NeuronCores.

Computation (per protein p, lengths len_p):
    scores[p,t,l] = sum_h pro[p,l,h] * term[t,h]          (T=512, L=1024, H=256)
    scores[p,t,l >= len_p] = -inf
    attn = softmax(scores, axis=l)
    out[p,t,h] = sum_l attn[p,t,l] * pro[p,l,h]
Returns (out, attn) like the reference.

Distribution: data-parallel over the 64 proteins, 8 per core. Proteins are
sorted by len and dealt round-robin so every core's slot j holds proteins of
near-identical length; the single SPMD program is specialized on the per-slot
[min_len, max_len] band, skipping all compute and DMA beyond max_len and
applying the -inf mask only inside the band (as a rank-1 matmul accumulated
straight into the scores PSUM).

Pipeline per protein:
  A: DMA pro slab; PE-transpose it to proT (fp32r transpose mode).
  B: per 128-row t-chunk: scores = termT.T @ proT (+ mask rank-1) in PSUM;
     reduce_max (negated) on DVE; exp with per-partition -max bias on ACT
     (accum_out gives the softmax sum for free); reciprocal; normalize in
     place; one 2MB attn DMA per protein.
  C: PE-transpose the normalized attn tiles to [l, t] layout.
  D: out = attnT.T @ pro accumulated over l-tiles; one out DMA per protein.

Matmuls and transposes run in fp32r (fp32 with 11-bit mantissa, full PE
rate). Everything feeding them is declared float32r and rounded (host RNE
pre-round for DMA-fed data, engine-output rounding for on-chip producers).
out is computed from the same rounded attn values that are written to DRAM,
so the two outputs are mutually consistent.
"""

import contextlib
import math

import numpy as np

import concourse.bass as bass
import concourse.tile as tile
from concourse import bacc, mybir
from concourse.bass_utils import run_bass_kernel_spmd

PRO_NUM, MAX_LEN, HID = 64, 1024, 256
TERM_NUM = 512
NCORES = 8
SLOTS = PRO_NUM // NCORES  # 8 proteins per core
MASK_VAL = -1.0e30
L_PAD = 32  # slot width granularity

DT = mybir.dt.float32
DTR = mybir.dt.float32r  # full-rate fp32 matmul mode (11-bit mantissa)
F32 = mybir.dt.float32
AF = mybir.ActivationFunctionType
AX = mybir.AxisListType


def _ceil_to(x, m):
    return min(MAX_LEN, ((x + m - 1) // m) * m)


def round_fp32r(x: np.ndarray) -> np.ndarray:
    """RNE-round fp32 to fp32r (11-bit mantissa), matching walrus
    fp32_to_fp32r."""
    b = x.astype(np.float32).view(np.uint32)
    low = b & np.uint32(0xFFF)
    hi = b & ~np.uint32(0xFFF)
    lsb = (hi >> np.uint32(12)) & np.uint32(1)
    round_up = (low > 0x800) | ((low == 0x800) & (lsb == 1))
    hi = hi + (round_up.astype(np.uint32) << np.uint32(12))
    return hi.view(np.float32)


ABLATE = set()


def _build_program(L, CLO=None, loop_reps=1):
    """Trace + compile the SPMD program. L[j] = padded max len of slot j,
    CLO[j] = min len of slot j (both identical on all cores). loop_reps>1
    wraps the body in a hardware loop (benchmarking only)."""
    if CLO is None:
        CLO = [0] * SLOTS
    nc = bacc.Bacc("TRN2", target_bir_lowering=False, debug=False)

    pro_in = nc.dram_tensor(
        "pro", [SLOTS, MAX_LEN, HID], DTR, kind="ExternalInput"
    ).ap()
    proT_in = nc.dram_tensor(
        "proT", [SLOTS, HID, MAX_LEN], DTR, kind="ExternalInput"
    ).ap()
    termT_in = nc.dram_tensor(
        "termT", [HID, TERM_NUM], DTR, kind="ExternalInput"
    ).ap()
    mask_in = nc.dram_tensor(
        "mask", [1, SLOTS * MAX_LEN], DTR, kind="ExternalInput"
    ).ap()
    ident_in = nc.dram_tensor("ident", [128, 128], DTR, kind="ExternalInput").ap()
    ones_in = nc.dram_tensor("ones", [1, TERM_NUM], DTR, kind="ExternalInput").ap()
    out_out = nc.dram_tensor(
        "out", [SLOTS, TERM_NUM, HID], DT, kind="ExternalOutput"
    ).ap()
    attn_out = nc.dram_tensor(
        "attn", [SLOTS, TERM_NUM, MAX_LEN], DT, kind="ExternalOutput"
    ).ap()

    NK = HID // 128  # 2 k-tiles over hidden
    NTC = TERM_NUM // 128  # 4 t-chunks

    with tile.TileContext(nc) as tc:
        with (
            tc.tile_pool(name="const", bufs=1) as const_pool,
            tc.tile_pool(name="pro", bufs=2) as pro_pool,
            tc.tile_pool(name="proT", bufs=2) as proT_pool,
            tc.tile_pool(name="attn", bufs=1) as attn_pool,
            tc.tile_pool(name="attnT", bufs=2) as attnT_pool,
            tc.tile_pool(name="stat", bufs=4) as stat_pool,
            tc.tile_pool(name="outp", bufs=1) as out_pool,
            tc.tile_pool(name="ps_s", bufs=2, space="PSUM") as psum_s,
            tc.tile_pool(name="ps_t", bufs=2, space="PSUM") as psum_t,
            tc.tile_pool(name="ps_o", bufs=2, space="PSUM") as psum_o,
        ):
            # --- constants ---
            termT_sb = const_pool.tile([128, NK, TERM_NUM], DTR)
            nc.sync.dma_start(
                termT_sb[:], termT_in.rearrange("(k p) t -> p k t", p=128)
            )
            ident_sb = const_pool.tile([128, 128], DTR)
            nc.sync.dma_start(ident_sb[:], ident_in[:])
            ones_sb = const_pool.tile([1, TERM_NUM], DTR)
            nc.sync.dma_start(ones_sb[:], ones_in[:])
            mask_all = const_pool.tile([1, SLOTS * MAX_LEN], DTR)
            nc.sync.dma_start(mask_all[:], mask_in[:])

            rep_ctx = (
                tc.For_i(0, loop_reps, 1)
                if loop_reps > 1
                else contextlib.nullcontext()
            )
            with rep_ctx:
                o_all = out_pool.tile([128, SLOTS, NTC, HID], DT, tag="out")
                for j in range(SLOTS):
                    Lj = L[j]
                    # 32-align the mask band start (ISA min width for fp32r)
                    clo = min(max(0, CLO[j]), Lj)
                    if clo < Lj:
                        clo = (min(clo, Lj - 32) // 32) * 32
                    nlt = math.ceil(Lj / 128)  # l-tiles of 128
                    lsz = [128] * (nlt - 1) + [Lj - 128 * (nlt - 1)]

                    # --- stage A: load pro (for mm2) and proT (for scores) ---
                    pro_sb = pro_pool.tile([128, nlt, HID], DTR, tag="pro")
                    nc.sync.dma_start(
                        pro_sb[:],
                        pro_in[j, 0 : nlt * 128, :].rearrange(
                            "(lt p) h -> p lt h", p=128
                        ),
                    )
                    proT_sb = proT_pool.tile([128, NK, nlt * 128], DTR, tag="proT")
                    nc.sync.dma_start(
                        proT_sb[:],
                        proT_in[j, :, 0 : nlt * 128].rearrange(
                            "(k p) l -> p k l", p=128
                        ),
                    )

                    recip_j = stat_pool.tile([128, NTC], DT, tag="recip")

                    # --- stage B: scores [t,l], softmax, attn ---
                    l_chunks = []
                    c0 = 0
                    while c0 < Lj:
                        c1 = min(c0 + 512, Lj)
                        l_chunks.append((c0, c1))
                        c0 = c1
                    # even/odd persistent tiles: ascending Lj means the
                    # tail [Lj, MAX_LEN) of this slot is already zero from
                    # its previous occupant; memset only on first use
                    a_tl = attn_pool.tile(
                        [128, NTC, MAX_LEN], DTR, tag=f"atl{j % 2}"
                    )
                    a_nm = attn_pool.tile(
                        [128, NTC, MAX_LEN], DT, tag=f"anm{j % 2}"
                    )
                    for tci in range(NTC):
                        t0 = tci * 128
                        ps_score = psum_s.tile([128, 1024], F32, tag="scores")
                        for k in range(NK if "mm1" not in ABLATE else 0):
                            for ci, (c0, c1) in enumerate(l_chunks):
                                last = ci == len(l_chunks) - 1
                                nc.tensor.matmul(
                                    ps_score[:, c0:c1],
                                    lhsT=termT_sb[:, k, t0 : t0 + 128],
                                    rhs=proT_sb[:, k, c0:c1],
                                    start=(k == 0),
                                    stop=(k == NK - 1 and clo >= Lj and last),
                                    skip_group_check=True,
                                )
                        if clo < Lj:
                            # rank-1 masking only over the [clo, Lj) band,
                            # chunked at PSUM bank boundaries
                            m0 = clo
                            while m0 < Lj:
                                m1 = min((m0 // 512 + 1) * 512, Lj)
                                nc.tensor.matmul(
                                    ps_score[:, m0:m1],
                                    lhsT=ones_sb[0:1, 0:128],
                                    rhs=mask_all[0:1, j * MAX_LEN + m0 : j * MAX_LEN + m1],
                                    start=False,
                                    stop=(m1 == Lj),
                                    skip_group_check=True,
                                )
                                m0 = m1
                        if "softmax" in ABLATE:
                            nc.scalar.copy(a_tl[:, tci, 0:Lj], ps_score[:, 0:Lj])
                            continue
                        negm = stat_pool.tile([128, 1], DT, tag="negm")
                        nc.vector.reduce_max(
                            negm[:], ps_score[:, 0:Lj], axis=AX.X, negate=True
                        )
                        ssum = stat_pool.tile([128, 1], DT, tag="ssum")
                        nc.scalar.activation(
                            a_tl[:, tci, 0:Lj],
                            ps_score[:, 0:Lj],
                            AF.Exp,
                            bias=negm[:, 0:1],
                            scale=1.0,
                            accum_out=ssum[:],
                        )
                        if Lj < MAX_LEN and j < 2:
                            nc.gpsimd.memset(a_nm[:, tci, Lj:MAX_LEN], 0.0)
                        nc.vector.reciprocal(recip_j[:, tci : tci + 1], ssum[:])
                        nc.vector.tensor_scalar_mul(
                            a_nm[:, tci, 0:Lj],
                            a_tl[:, tci, 0:Lj].bitcast(F32),
                            recip_j[:, tci : tci + 1],
                        )
                    if "attndma" not in ABLATE:
                        nc.sync.dma_start(
                            attn_out[j].rearrange("(tc p) l -> p tc l", p=128),
                            a_nm[:],
                        )

                    if "cd" in ABLATE:
                        continue
                    # --- stage C: transpose attn to [l, t] ---
                    a_ltT = attnT_pool.tile([128, SLOTS, TERM_NUM], DTR, tag="alt")
                    for li in range(nlt):
                        ls = lsz[li]
                        l0 = li * 128
                        ps_tt = psum_t.tile([128, 512], DTR, tag="tp")
                        for tci in range(NTC):
                            nc.tensor.transpose(
                                ps_tt[0:ls, tci * 128 : (tci + 1) * 128],
                                a_tl[:, tci, l0 : l0 + ls],
                                ident_sb[:, :],
                            )
                        nc.vector.tensor_copy(a_ltT[0:ls, li, :], ps_tt[0:ls, :])

                    # --- stage D: out = attnT.T @ pro ---
                    for tci in range(NTC):
                        t0 = tci * 128
                        ps_out = psum_o.tile([128, HID], F32, tag="psout")
                        for li in range(nlt):
                            ls = lsz[li]
                            nc.tensor.matmul(
                                ps_out[:, :],
                                lhsT=a_ltT[0:ls, li, t0 : t0 + 128],
                                rhs=pro_sb[0:ls, li, :],
                                start=(li == 0),
                                stop=(li == nlt - 1),
                            )
                        nc.scalar.mul(
                            o_all[:, j, tci, :], ps_out[:, :],
                            recip_j[:, tci : tci + 1],
                        )
                if "cd" not in ABLATE:
                    nc.sync.dma_start(
                        out_out.rearrange("j (tc p) h -> p j tc h", p=128),
                        o_all[:],
                    )

    nc.compile()
    return nc


def prepare(term_encoding, pro_encoding, pro_lens):
    """Host-side prep: sorted round-robin deal, per-slot widths, per-core
    input maps. Returns (L, CLO, order, in_maps)."""
    term = np.ascontiguousarray(np.asarray(term_encoding, dtype=np.float32))
    pro = np.ascontiguousarray(np.asarray(pro_encoding, dtype=np.float32))
    lens = np.asarray(pro_lens).astype(np.int64)
    assert term.shape == (TERM_NUM, HID) and pro.shape == (PRO_NUM, MAX_LEN, HID)

    # sorted round-robin deal: slot j on core c gets protein order[j*8+c]
    order = np.argsort(lens, kind="stable")
    L, CLO = [], []
    for j in range(SLOTS):
        group = order[j * NCORES : (j + 1) * NCORES]
        L.append(max(L_PAD, _ceil_to(int(lens[group].max()), L_PAD)))
        CLO.append(min(int(lens[group].min()), L[-1]))

    NKH = HID // 128
    termT = round_fp32r(np.ascontiguousarray(term.T))
    ident = np.eye(128, dtype=np.float32)
    pro_r = round_fp32r(pro)
    iota = np.arange(MAX_LEN)

    in_maps = []
    for c in range(NCORES):
        idx = [int(order[j * NCORES + c]) for j in range(SLOTS)]
        mask = np.where(
            iota[None, :] >= lens[idx][:, None], np.float32(MASK_VAL), np.float32(0)
        ).astype(np.float32)
        in_maps.append(
            {
                "pro": np.ascontiguousarray(pro_r[idx]),
                "proT": np.ascontiguousarray(pro_r[idx].transpose(0, 2, 1)),
                "termT": termT,
                "mask": round_fp32r(mask).reshape(1, SLOTS * MAX_LEN),
                "ident": ident,
                "ones": np.ones((1, TERM_NUM), dtype=np.float32),
            }
        )
    return L, CLO, order, in_maps


def kernel(term_encoding, pro_encoding, pro_lens):
    L, CLO, order, in_maps = prepare(term_encoding, pro_encoding, pro_lens)
    nc = _build_program(L, CLO)
    res = run_bass_kernel_spmd(nc, in_maps, list(range(NCORES)))

    out_full = np.empty((PRO_NUM, TERM_NUM, HID), dtype=np.float32)
    attn_full = np.empty((PRO_NUM, TERM_NUM, MAX_LEN), dtype=np.float32)
    for c in range(NCORES):
        r = res.results[c]
        for j in range(SLOTS):
            p = int(order[j * NCORES + c])
            out_full[p] = r["out"][j]
            attn_full[p] = r["attn"][j]
    return out_full, attn_full


# revision 46
# speedup vs baseline: 1.1246x; 1.1246x over previous
"""Masked per-protein attention (sparse_attention) on 8 trn2 NeuronCores.

Computation (per protein p, lengths len_p):
    scores[p,t,l] = sum_h pro[p,l,h] * term[t,h]          (T=512, L=1024, H=256)
    scores[p,t,l >= len_p] = -inf
    attn = softmax(scores, axis=l)
    out[p,t,h] = sum_l attn[p,t,l] * pro[p,l,h]
Returns (out, attn) like the reference.

Distribution: data-parallel over the 64 proteins, 8 per core. Proteins are
sorted by len and dealt round-robin so every core's slot j holds proteins of
near-identical length; the single SPMD program is specialized on the per-slot
[min_len, max_len] band, skipping all compute and DMA beyond max_len and
applying the -inf mask only inside the band (as a rank-1 matmul accumulated
straight into the scores PSUM).

Pipeline per protein:
  A: DMA pro slab; PE-transpose it to proT (fp32r transpose mode).
  B: per 128-row t-chunk: scores = termT.T @ proT (+ mask rank-1) in PSUM;
     reduce_max (negated) on DVE; exp with per-partition -max bias on ACT
     (accum_out gives the softmax sum for free); reciprocal; normalize in
     place; one 2MB attn DMA per protein.
  C: PE-transpose the normalized attn tiles to [l, t] layout.
  D: out = attnT.T @ pro accumulated over l-tiles; one out DMA per protein.

Matmuls and transposes run in fp32r (fp32 with 11-bit mantissa, full PE
rate). Everything feeding them is declared float32r and rounded (host RNE
pre-round for DMA-fed data, engine-output rounding for on-chip producers).
out is computed from the same rounded attn values that are written to DRAM,
so the two outputs are mutually consistent.
"""

import contextlib
import math

import numpy as np

import concourse.bass as bass
import concourse.tile as tile
from concourse import bacc, mybir
from concourse.bass_utils import run_bass_kernel_spmd

PRO_NUM, MAX_LEN, HID = 64, 1024, 256
TERM_NUM = 512
NCORES = 8
SLOTS = PRO_NUM // NCORES  # 8 proteins per core
MASK_VAL = -1.0e30
L_PAD = 32  # slot width granularity

DT = mybir.dt.float32
DTR = mybir.dt.float32r  # full-rate fp32 matmul mode (11-bit mantissa)
F32 = mybir.dt.float32
AF = mybir.ActivationFunctionType
AX = mybir.AxisListType


def _ceil_to(x, m):
    return min(MAX_LEN, ((x + m - 1) // m) * m)


def round_fp32r(x: np.ndarray) -> np.ndarray:
    """RNE-round fp32 to fp32r (11-bit mantissa), matching walrus
    fp32_to_fp32r."""
    b = x.astype(np.float32).view(np.uint32)
    low = b & np.uint32(0xFFF)
    hi = b & ~np.uint32(0xFFF)
    lsb = (hi >> np.uint32(12)) & np.uint32(1)
    round_up = (low > 0x800) | ((low == 0x800) & (lsb == 1))
    hi = hi + (round_up.astype(np.uint32) << np.uint32(12))
    return hi.view(np.float32)


ABLATE = set()


def _build_program(L, CLO=None, loop_reps=1):
    """Trace + compile the SPMD program. L[j] = padded max len of slot j,
    CLO[j] = min len of slot j (both identical on all cores). loop_reps>1
    wraps the body in a hardware loop (benchmarking only)."""
    if CLO is None:
        CLO = [0] * SLOTS
    nc = bacc.Bacc("TRN2", target_bir_lowering=False, debug=False)

    pro_in = nc.dram_tensor(
        "pro", [SLOTS, MAX_LEN, HID], DTR, kind="ExternalInput"
    ).ap()
    proT_in = nc.dram_tensor(
        "proT", [SLOTS, HID, MAX_LEN], DTR, kind="ExternalInput"
    ).ap()
    termT_in = nc.dram_tensor(
        "termT", [HID, TERM_NUM], DTR, kind="ExternalInput"
    ).ap()
    mask_in = nc.dram_tensor(
        "mask", [1, SLOTS * MAX_LEN], DTR, kind="ExternalInput"
    ).ap()
    ident_in = nc.dram_tensor("ident", [128, 128], DTR, kind="ExternalInput").ap()
    ones_in = nc.dram_tensor("ones", [1, TERM_NUM], DTR, kind="ExternalInput").ap()
    out_out = nc.dram_tensor(
        "out", [SLOTS, TERM_NUM, HID], DT, kind="ExternalOutput"
    ).ap()
    attn_out = nc.dram_tensor(
        "attn", [SLOTS, TERM_NUM, MAX_LEN], DT, kind="ExternalOutput"
    ).ap()

    NK = HID // 128  # 2 k-tiles over hidden
    NTC = TERM_NUM // 128  # 4 t-chunks

    with tile.TileContext(nc) as tc:
        with (
            tc.tile_pool(name="const", bufs=1) as const_pool,
            tc.tile_pool(name="pro", bufs=2) as pro_pool,
            tc.tile_pool(name="proT", bufs=2) as proT_pool,
            tc.tile_pool(name="attn", bufs=1) as attn_pool,
            tc.tile_pool(name="attnT", bufs=2) as attnT_pool,
            tc.tile_pool(name="stat", bufs=4) as stat_pool,
            tc.tile_pool(name="outp", bufs=1) as out_pool,
            tc.tile_pool(name="ps_s", bufs=2, space="PSUM") as psum_s,
            tc.tile_pool(name="ps_t", bufs=2, space="PSUM") as psum_t,
            tc.tile_pool(name="ps_o", bufs=2, space="PSUM") as psum_o,
        ):
            # --- constants ---
            termT_sb = const_pool.tile([128, NK, TERM_NUM], DTR)
            nc.sync.dma_start(
                termT_sb[:], termT_in.rearrange("(k p) t -> p k t", p=128)
            )
            ident_sb = const_pool.tile([128, 128], DTR)
            nc.sync.dma_start(ident_sb[:], ident_in[:])
            ones_sb = const_pool.tile([1, TERM_NUM], DTR)
            nc.sync.dma_start(ones_sb[:], ones_in[:])
            mask_all = const_pool.tile([1, SLOTS * MAX_LEN], DTR)
            nc.sync.dma_start(mask_all[:], mask_in[:])

            rep_ctx = (
                tc.For_i(0, loop_reps, 1)
                if loop_reps > 1
                else contextlib.nullcontext()
            )
            with rep_ctx:
                o_all = out_pool.tile([128, SLOTS, NTC, HID], DT, tag="out")
                for j in range(SLOTS):
                    Lj = L[j]
                    # 32-align the mask band start (ISA min width for fp32r)
                    clo = min(max(0, CLO[j]), Lj)
                    if clo < Lj:
                        clo = (min(clo, Lj - 32) // 32) * 32
                    nlt = math.ceil(Lj / 128)  # l-tiles of 128
                    lsz = [128] * (nlt - 1) + [Lj - 128 * (nlt - 1)]

                    # --- stage A: load pro (for mm2) and proT (for scores) ---
                    pro_sb = pro_pool.tile([128, nlt, HID], DTR, tag="pro")
                    nc.sync.dma_start(
                        pro_sb[:],
                        pro_in[j, 0 : nlt * 128, :].rearrange(
                            "(lt p) h -> p lt h", p=128
                        ),
                    )
                    proT_sb = proT_pool.tile([128, NK, nlt * 128], DTR, tag="proT")
                    nc.sync.dma_start(
                        proT_sb[:],
                        proT_in[j, :, 0 : nlt * 128].rearrange(
                            "(k p) l -> p k l", p=128
                        ),
                    )

                    recip_j = stat_pool.tile([128, NTC], DT, tag="recip")

                    # --- stage B: scores [t,l], softmax, attn ---
                    l_chunks = []
                    c0 = 0
                    while c0 < Lj:
                        c1 = min(c0 + 512, Lj)
                        l_chunks.append((c0, c1))
                        c0 = c1
                    # even/odd persistent tiles: ascending Lj means the
                    # tail [Lj, MAX_LEN) of this slot is already zero from
                    # its previous occupant; memset only on first use
                    a_tl = attn_pool.tile(
                        [128, NTC, MAX_LEN], DTR, tag=f"atl{j % 2}"
                    )
                    for tci in range(NTC):
                        t0 = tci * 128
                        ps_score = psum_s.tile([128, 1024], F32, tag="scores")
                        for k in range(NK if "mm1" not in ABLATE else 0):
                            for ci, (c0, c1) in enumerate(l_chunks):
                                last = ci == len(l_chunks) - 1
                                nc.tensor.matmul(
                                    ps_score[:, c0:c1],
                                    lhsT=termT_sb[:, k, t0 : t0 + 128],
                                    rhs=proT_sb[:, k, c0:c1],
                                    start=(k == 0),
                                    stop=(k == NK - 1 and clo >= Lj and last),
                                    skip_group_check=True,
                                )
                        if clo < Lj:
                            # rank-1 masking only over the [clo, Lj) band,
                            # chunked at PSUM bank boundaries
                            m0 = clo
                            while m0 < Lj:
                                m1 = min((m0 // 512 + 1) * 512, Lj)
                                nc.tensor.matmul(
                                    ps_score[:, m0:m1],
                                    lhsT=ones_sb[0:1, 0:128],
                                    rhs=mask_all[0:1, j * MAX_LEN + m0 : j * MAX_LEN + m1],
                                    start=False,
                                    stop=(m1 == Lj),
                                    skip_group_check=True,
                                )
                                m0 = m1
                        if "softmax" in ABLATE:
                            nc.scalar.copy(a_tl[:, tci, 0:Lj], ps_score[:, 0:Lj])
                            continue
                        negm = stat_pool.tile([128, 1], DT, tag="negm")
                        nc.vector.reduce_max(
                            negm[:], ps_score[:, 0:Lj], axis=AX.X, negate=True
                        )
                        ssum = stat_pool.tile([128, 1], DT, tag="ssum")
                        nc.scalar.activation(
                            a_tl[:, tci, 0:Lj],
                            ps_score[:, 0:Lj],
                            AF.Exp,
                            bias=negm[:, 0:1],
                            scale=1.0,
                            accum_out=ssum[:],
                        )
                        if Lj < MAX_LEN and j < 2:
                            nc.gpsimd.memset(
                                a_tl[:, tci, Lj:MAX_LEN].bitcast(F32), 0.0
                            )
                        nc.vector.reciprocal(recip_j[:, tci : tci + 1], ssum[:])
                        nc.vector.tensor_scalar_mul(
                            a_tl[:, tci, 0:Lj],
                            a_tl[:, tci, 0:Lj],
                            recip_j[:, tci : tci + 1],
                        )
                    if "attndma" not in ABLATE:
                        nc.sync.dma_start(
                            attn_out[j].rearrange("(tc p) l -> p tc l", p=128),
                            a_tl[:].bitcast(F32),
                        )

                    if "cd" in ABLATE:
                        continue
                    # --- stage C: transpose attn to [l, t] ---
                    a_ltT = attnT_pool.tile([128, SLOTS, TERM_NUM], DTR, tag="alt")
                    for li in range(nlt):
                        ls = lsz[li]
                        l0 = li * 128
                        ps_tt = psum_t.tile([128, 512], DTR, tag="tp")
                        for tci in range(NTC):
                            nc.tensor.transpose(
                                ps_tt[0:ls, tci * 128 : (tci + 1) * 128],
                                a_tl[:, tci, l0 : l0 + ls],
                                ident_sb[:, :],
                            )
                        nc.vector.tensor_copy(a_ltT[0:ls, li, :], ps_tt[0:ls, :])

                    # --- stage D: out = attnT.T @ pro ---
                    for tci in range(NTC):
                        t0 = tci * 128
                        ps_out = psum_o.tile([128, HID], F32, tag="psout")
                        for li in range(nlt):
                            ls = lsz[li]
                            nc.tensor.matmul(
                                ps_out[:, :],
                                lhsT=a_ltT[0:ls, li, t0 : t0 + 128],
                                rhs=pro_sb[0:ls, li, :],
                                start=(li == 0),
                                stop=(li == nlt - 1),
                            )
                        nc.scalar.copy(o_all[:, j, tci, :], ps_out[:, :])
                if "cd" not in ABLATE:
                    nc.sync.dma_start(
                        out_out.rearrange("j (tc p) h -> p j tc h", p=128),
                        o_all[:],
                    )

    nc.compile()
    return nc


def prepare(term_encoding, pro_encoding, pro_lens):
    """Host-side prep: sorted round-robin deal, per-slot widths, per-core
    input maps. Returns (L, CLO, order, in_maps)."""
    term = np.ascontiguousarray(np.asarray(term_encoding, dtype=np.float32))
    pro = np.ascontiguousarray(np.asarray(pro_encoding, dtype=np.float32))
    lens = np.asarray(pro_lens).astype(np.int64)
    assert term.shape == (TERM_NUM, HID) and pro.shape == (PRO_NUM, MAX_LEN, HID)

    # sorted round-robin deal: slot j on core c gets protein order[j*8+c]
    order = np.argsort(lens, kind="stable")
    L, CLO = [], []
    for j in range(SLOTS):
        group = order[j * NCORES : (j + 1) * NCORES]
        L.append(max(L_PAD, _ceil_to(int(lens[group].max()), L_PAD)))
        CLO.append(min(int(lens[group].min()), L[-1]))

    NKH = HID // 128
    termT = round_fp32r(np.ascontiguousarray(term.T))
    ident = np.eye(128, dtype=np.float32)
    pro_r = round_fp32r(pro)
    iota = np.arange(MAX_LEN)

    in_maps = []
    for c in range(NCORES):
        idx = [int(order[j * NCORES + c]) for j in range(SLOTS)]
        mask = np.where(
            iota[None, :] >= lens[idx][:, None], np.float32(MASK_VAL), np.float32(0)
        ).astype(np.float32)
        in_maps.append(
            {
                "pro": np.ascontiguousarray(pro_r[idx]),
                "proT": np.ascontiguousarray(pro_r[idx].transpose(0, 2, 1)),
                "termT": termT,
                "mask": round_fp32r(mask).reshape(1, SLOTS * MAX_LEN),
                "ident": ident,
                "ones": np.ones((1, TERM_NUM), dtype=np.float32),
            }
        )
    return L, CLO, order, in_maps


def kernel(term_encoding, pro_encoding, pro_lens):
    L, CLO, order, in_maps = prepare(term_encoding, pro_encoding, pro_lens)
    nc = _build_program(L, CLO)
    res = run_bass_kernel_spmd(nc, in_maps, list(range(NCORES)))

    out_full = np.empty((PRO_NUM, TERM_NUM, HID), dtype=np.float32)
    attn_full = np.empty((PRO_NUM, TERM_NUM, MAX_LEN), dtype=np.float32)
    for c in range(NCORES):
        r = res.results[c]
        for j in range(SLOTS):
            p = int(order[j * NCORES + c])
            out_full[p] = r["out"][j]
            attn_full[p] = r["attn"][j]
    return out_full, attn_full


# revision 47
# speedup vs baseline: 1.2084x; 1.0744x over previous
"""Masked per-protein attention (sparse_attention) on 8 trn2 NeuronCores.

Computation (per protein p, lengths len_p):
    scores[p,t,l] = sum_h pro[p,l,h] * term[t,h]          (T=512, L=1024, H=256)
    scores[p,t,l >= len_p] = -inf
    attn = softmax(scores, axis=l)
    out[p,t,h] = sum_l attn[p,t,l] * pro[p,l,h]
Returns (out, attn) like the reference.

Distribution: data-parallel over the 64 proteins, 8 per core. Proteins are
sorted by len and dealt round-robin so every core's slot j holds proteins of
near-identical length; the single SPMD program is specialized on the per-slot
[min_len, max_len] band, skipping all compute and DMA beyond max_len and
applying the -inf mask only inside the band (as a rank-1 matmul accumulated
straight into the scores PSUM).

Pipeline per protein:
  A: DMA pro slab; PE-transpose it to proT (fp32r transpose mode).
  B: per 128-row t-chunk: scores = termT.T @ proT (+ mask rank-1) in PSUM;
     reduce_max (negated) on DVE; exp with per-partition -max bias on ACT
     (accum_out gives the softmax sum for free); reciprocal; normalize in
     place; one 2MB attn DMA per protein.
  C: PE-transpose the normalized attn tiles to [l, t] layout.
  D: out = attnT.T @ pro accumulated over l-tiles; one out DMA per protein.

Matmuls and transposes run in fp32r (fp32 with 11-bit mantissa, full PE
rate). Everything feeding them is declared float32r and rounded (host RNE
pre-round for DMA-fed data, engine-output rounding for on-chip producers).
out is computed from the same rounded attn values that are written to DRAM,
so the two outputs are mutually consistent.
"""

import contextlib
import math

import numpy as np

import concourse.bass as bass
import concourse.tile as tile
from concourse import bacc, mybir
from concourse.bass_utils import run_bass_kernel_spmd

PRO_NUM, MAX_LEN, HID = 64, 1024, 256
TERM_NUM = 512
NCORES = 8
SLOTS = PRO_NUM // NCORES  # 8 proteins per core
MASK_VAL = -1.0e30
L_PAD = 32  # slot width granularity

DT = mybir.dt.float32
DTR = mybir.dt.float32r  # full-rate fp32 matmul mode (11-bit mantissa)
F32 = mybir.dt.float32
AF = mybir.ActivationFunctionType
AX = mybir.AxisListType


def _ceil_to(x, m):
    return min(MAX_LEN, ((x + m - 1) // m) * m)


def round_fp32r(x: np.ndarray) -> np.ndarray:
    """RNE-round fp32 to fp32r (11-bit mantissa), matching walrus
    fp32_to_fp32r."""
    b = x.astype(np.float32).view(np.uint32)
    low = b & np.uint32(0xFFF)
    hi = b & ~np.uint32(0xFFF)
    lsb = (hi >> np.uint32(12)) & np.uint32(1)
    round_up = (low > 0x800) | ((low == 0x800) & (lsb == 1))
    hi = hi + (round_up.astype(np.uint32) << np.uint32(12))
    return hi.view(np.float32)


ABLATE = set()


def _build_program(L, CLO=None, loop_reps=1):
    """Trace + compile the SPMD program. L[j] = padded max len of slot j,
    CLO[j] = min len of slot j (both identical on all cores). loop_reps>1
    wraps the body in a hardware loop (benchmarking only)."""
    if CLO is None:
        CLO = [0] * SLOTS
    nc = bacc.Bacc("TRN2", target_bir_lowering=False, debug=False)

    pro_in = nc.dram_tensor(
        "pro", [SLOTS, MAX_LEN, HID], DTR, kind="ExternalInput"
    ).ap()
    proT_in = nc.dram_tensor(
        "proT", [SLOTS, HID, MAX_LEN], DTR, kind="ExternalInput"
    ).ap()
    termT_in = nc.dram_tensor(
        "termT", [HID, TERM_NUM], DTR, kind="ExternalInput"
    ).ap()
    mask_in = nc.dram_tensor(
        "mask", [1, SLOTS * MAX_LEN], DTR, kind="ExternalInput"
    ).ap()
    ident_in = nc.dram_tensor("ident", [128, 128], DTR, kind="ExternalInput").ap()
    ones_in = nc.dram_tensor("ones", [1, TERM_NUM], DTR, kind="ExternalInput").ap()
    out_out = nc.dram_tensor(
        "out", [SLOTS, TERM_NUM, HID], DT, kind="ExternalOutput"
    ).ap()
    attn_out = nc.dram_tensor(
        "attn", [SLOTS, TERM_NUM, MAX_LEN], DT, kind="ExternalOutput"
    ).ap()

    NK = HID // 128  # 2 k-tiles over hidden
    NTC = TERM_NUM // 128  # 4 t-chunks

    with tile.TileContext(nc) as tc:
        with (
            tc.tile_pool(name="const", bufs=1) as const_pool,
            tc.tile_pool(name="pro", bufs=2) as pro_pool,
            tc.tile_pool(name="proT", bufs=2) as proT_pool,
            tc.tile_pool(name="attn", bufs=1) as attn_pool,
            tc.tile_pool(name="attnT", bufs=2) as attnT_pool,
            tc.tile_pool(name="stat", bufs=4) as stat_pool,
            tc.tile_pool(name="outp", bufs=1) as out_pool,
            tc.tile_pool(name="ps_s", bufs=2, space="PSUM") as psum_s,
            tc.tile_pool(name="ps_t", bufs=2, space="PSUM") as psum_t,
            tc.tile_pool(name="ps_o", bufs=2, space="PSUM") as psum_o,
        ):
            # --- constants ---
            termT_sb = const_pool.tile([128, NK, TERM_NUM], DTR)
            nc.sync.dma_start(
                termT_sb[:], termT_in.rearrange("(k p) t -> p k t", p=128)
            )
            ident_sb = const_pool.tile([128, 128], DTR)
            nc.sync.dma_start(ident_sb[:], ident_in[:])
            ones_sb = const_pool.tile([1, TERM_NUM], DTR)
            nc.sync.dma_start(ones_sb[:], ones_in[:])
            mask_all = const_pool.tile([1, SLOTS * MAX_LEN], DTR)
            nc.sync.dma_start(mask_all[:], mask_in[:])

            rep_ctx = (
                tc.For_i(0, loop_reps, 1)
                if loop_reps > 1
                else contextlib.nullcontext()
            )
            with rep_ctx:
                o_all = out_pool.tile([128, SLOTS, NTC, HID], DT, tag="out")
                for j in range(SLOTS):
                    Lj = L[j]
                    # 32-align the mask band start (ISA min width for fp32r)
                    clo = min(max(0, CLO[j]), Lj)
                    if clo < Lj:
                        clo = (min(clo, Lj - 32) // 32) * 32
                    nlt = math.ceil(Lj / 128)  # l-tiles of 128
                    lsz = [128] * (nlt - 1) + [Lj - 128 * (nlt - 1)]

                    # --- stage A: load pro (for mm2) and proT (for scores) ---
                    pro_sb = pro_pool.tile([128, nlt, HID], DTR, tag="pro")
                    nc.gpsimd.dma_start(
                        pro_sb[:],
                        pro_in[j, 0 : nlt * 128, :].rearrange(
                            "(lt p) h -> p lt h", p=128
                        ),
                    )
                    proT_sb = proT_pool.tile([128, NK, nlt * 128], DTR, tag="proT")
                    nc.gpsimd.dma_start(
                        proT_sb[:],
                        proT_in[j, :, 0 : nlt * 128].rearrange(
                            "(k p) l -> p k l", p=128
                        ),
                    )

                    recip_j = stat_pool.tile([128, NTC], DT, tag="recip")

                    # --- stage B: scores [t,l], softmax, attn ---
                    l_chunks = []
                    c0 = 0
                    while c0 < Lj:
                        c1 = min(c0 + 512, Lj)
                        l_chunks.append((c0, c1))
                        c0 = c1
                    # even/odd persistent tiles: ascending Lj means the
                    # tail [Lj, MAX_LEN) of this slot is already zero from
                    # its previous occupant; memset only on first use
                    a_tl = attn_pool.tile(
                        [128, NTC, MAX_LEN], DTR, tag=f"atl{j % 2}"
                    )
                    for tci in range(NTC):
                        t0 = tci * 128
                        ps_score = psum_s.tile([128, 1024], F32, tag="scores")
                        for k in range(NK if "mm1" not in ABLATE else 0):
                            for ci, (c0, c1) in enumerate(l_chunks):
                                last = ci == len(l_chunks) - 1
                                nc.tensor.matmul(
                                    ps_score[:, c0:c1],
                                    lhsT=termT_sb[:, k, t0 : t0 + 128],
                                    rhs=proT_sb[:, k, c0:c1],
                                    start=(k == 0),
                                    stop=(k == NK - 1 and clo >= Lj and last),
                                    skip_group_check=True,
                                )
                        if clo < Lj:
                            # rank-1 masking only over the [clo, Lj) band,
                            # chunked at PSUM bank boundaries
                            m0 = clo
                            while m0 < Lj:
                                m1 = min((m0 // 512 + 1) * 512, Lj)
                                nc.tensor.matmul(
                                    ps_score[:, m0:m1],
                                    lhsT=ones_sb[0:1, 0:128],
                                    rhs=mask_all[0:1, j * MAX_LEN + m0 : j * MAX_LEN + m1],
                                    start=False,
                                    stop=(m1 == Lj),
                                    skip_group_check=True,
                                )
                                m0 = m1
                        if "softmax" in ABLATE:
                            nc.scalar.copy(a_tl[:, tci, 0:Lj], ps_score[:, 0:Lj])
                            continue
                        negm = stat_pool.tile([128, 1], DT, tag="negm")
                        nc.vector.reduce_max(
                            negm[:], ps_score[:, 0:Lj], axis=AX.X, negate=True
                        )
                        ssum = stat_pool.tile([128, 1], DT, tag="ssum")
                        nc.scalar.activation(
                            a_tl[:, tci, 0:Lj],
                            ps_score[:, 0:Lj],
                            AF.Exp,
                            bias=negm[:, 0:1],
                            scale=1.0,
                            accum_out=ssum[:],
                        )
                        if Lj < MAX_LEN and j < 2:
                            nc.gpsimd.memset(
                                a_tl[:, tci, Lj:MAX_LEN].bitcast(F32), 0.0
                            )
                        nc.vector.reciprocal(recip_j[:, tci : tci + 1], ssum[:])
                        nc.vector.tensor_scalar_mul(
                            a_tl[:, tci, 0:Lj],
                            a_tl[:, tci, 0:Lj],
                            recip_j[:, tci : tci + 1],
                        )
                    if "attndma" not in ABLATE:
                        nc.sync.dma_start(
                            attn_out[j].rearrange("(tc p) l -> p tc l", p=128),
                            a_tl[:].bitcast(F32),
                        )

                    if "cd" in ABLATE:
                        continue
                    # --- stage C: transpose attn to [l, t] ---
                    a_ltT = attnT_pool.tile([128, SLOTS, TERM_NUM], DTR, tag="alt")
                    for li in range(nlt):
                        ls = lsz[li]
                        l0 = li * 128
                        ps_tt = psum_t.tile([128, 512], DTR, tag="tp")
                        for tci in range(NTC):
                            nc.tensor.transpose(
                                ps_tt[0:ls, tci * 128 : (tci + 1) * 128],
                                a_tl[:, tci, l0 : l0 + ls],
                                ident_sb[:, :],
                            )
                        nc.vector.tensor_copy(a_ltT[0:ls, li, :], ps_tt[0:ls, :])

                    # --- stage D: out = attnT.T @ pro ---
                    for tci in range(NTC):
                        t0 = tci * 128
                        ps_out = psum_o.tile([128, HID], F32, tag="psout")
                        for li in range(nlt):
                            ls = lsz[li]
                            nc.tensor.matmul(
                                ps_out[:, :],
                                lhsT=a_ltT[0:ls, li, t0 : t0 + 128],
                                rhs=pro_sb[0:ls, li, :],
                                start=(li == 0),
                                stop=(li == nlt - 1),
                            )
                        nc.scalar.copy(o_all[:, j, tci, :], ps_out[:, :])
                if "cd" not in ABLATE:
                    nc.sync.dma_start(
                        out_out.rearrange("j (tc p) h -> p j tc h", p=128),
                        o_all[:],
                    )

    nc.compile()
    return nc


def prepare(term_encoding, pro_encoding, pro_lens):
    """Host-side prep: sorted round-robin deal, per-slot widths, per-core
    input maps. Returns (L, CLO, order, in_maps)."""
    term = np.ascontiguousarray(np.asarray(term_encoding, dtype=np.float32))
    pro = np.ascontiguousarray(np.asarray(pro_encoding, dtype=np.float32))
    lens = np.asarray(pro_lens).astype(np.int64)
    assert term.shape == (TERM_NUM, HID) and pro.shape == (PRO_NUM, MAX_LEN, HID)

    # sorted round-robin deal: slot j on core c gets protein order[j*8+c]
    order = np.argsort(lens, kind="stable")
    L, CLO = [], []
    for j in range(SLOTS):
        group = order[j * NCORES : (j + 1) * NCORES]
        L.append(max(L_PAD, _ceil_to(int(lens[group].max()), L_PAD)))
        CLO.append(min(int(lens[group].min()), L[-1]))

    NKH = HID // 128
    termT = round_fp32r(np.ascontiguousarray(term.T))
    ident = np.eye(128, dtype=np.float32)
    pro_r = round_fp32r(pro)
    iota = np.arange(MAX_LEN)

    in_maps = []
    for c in range(NCORES):
        idx = [int(order[j * NCORES + c]) for j in range(SLOTS)]
        mask = np.where(
            iota[None, :] >= lens[idx][:, None], np.float32(MASK_VAL), np.float32(0)
        ).astype(np.float32)
        in_maps.append(
            {
                "pro": np.ascontiguousarray(pro_r[idx]),
                "proT": np.ascontiguousarray(pro_r[idx].transpose(0, 2, 1)),
                "termT": termT,
                "mask": round_fp32r(mask).reshape(1, SLOTS * MAX_LEN),
                "ident": ident,
                "ones": np.ones((1, TERM_NUM), dtype=np.float32),
            }
        )
    return L, CLO, order, in_maps


def kernel(term_encoding, pro_encoding, pro_lens):
    L, CLO, order, in_maps = prepare(term_encoding, pro_encoding, pro_lens)
    nc = _build_program(L, CLO)
    res = run_bass_kernel_spmd(nc, in_maps, list(range(NCORES)))

    out_full = np.empty((PRO_NUM, TERM_NUM, HID), dtype=np.float32)
    attn_full = np.empty((PRO_NUM, TERM_NUM, MAX_LEN), dtype=np.float32)
    for c in range(NCORES):
        r = res.results[c]
        for j in range(SLOTS):
            p = int(order[j * NCORES + c])
            out_full[p] = r["out"][j]
            attn_full[p] = r["attn"][j]
    return out_full, attn_full
